# revision 1
# baseline (speedup 1.0000x reference)
"""PostCrossAttention Trainium2 kernel.

Reference computation (per batch b):
    qh = (q @ Wq.T)  split into H=8 heads of dh=96   -> [H, N, 96]
    kh = (k @ Wk.T)  likewise
    vh = (v @ Wv.T)  split into H=8 heads of dv=64   -> [H, N, 64]
    S  = qh @ kh.T * SCALE          (SCALE = (256//8)**-0.5 = 32**-0.5)
    A  = softmax(S, axis=-1)
    A  = A * m / (H * sum(m, -1, keepdims))
    x  = A @ vh   -> concat heads -> [N, 512]

Sharding: 8 cores = 4 batches x 2 head-groups (4 heads each).
Each core receives host-pre-transposed bf16 operands and computes its
[2048, 256] slice of the output.

Device dataflow (per core, per head):
    S.T[j,i] = Kp @ Qp.T   (via lhsT=KpT tile, rhs=QpT, K=96 contraction)
    expS.T   = exp(S.T * SCALE)            (ACT, from PSUM, bf16 out)
    B.T      = expS.T * masks.T            (DVE, bf16)
    U.T[0:64 ,i] += Vp[jt].T @ B.T[jt]     (PE, accumulated over jt)
    U.T[64:65,i] += ones.T   @ expS.T[jt]  (PE, = sumexp row)
    U = transpose(U.T)  (PE, 128x65 tiles) then
    x[i, d] = U[i, d] / (8 * summ[i] * sumexp[i])   (DVE)
"""

import sys

for _p in ("/opt/trn_rl_repo",):
    if _p not in sys.path:
        sys.path.insert(0, _p)

from contextlib import ExitStack

import ml_dtypes
import numpy as np

import concourse.bass as bass
import concourse.bacc as bacc_mod
import concourse.bass_utils as _bu

# walrus's LDWEIGHTS dedup pass is off by default; repeated stationary
# reloads (ones / Vp / KpT reused across matmuls) serialize the PE here,
# so turn it on (correctness is covered by the test harness).
if not getattr(_bu, "_ldw_opt_patched", False):
    _orig_run_command = _bu.run_command

    def _run_command_ldw(argv, **kwargs):
        import os as _os
        if _os.environ.get("BASS_LDW_OPT", "0") == "1":
            argv = [a.replace("--enable-ldw-opt=false", "--enable-ldw-opt=true")
                    if isinstance(a, str) else a for a in argv]
        return _orig_run_command(argv, **kwargs)

    _bu.run_command = _run_command_ldw
    _bu._ldw_opt_patched = True
import concourse.mybir as mybir
import concourse.tile as tile
from concourse.masks import make_identity

F32 = mybir.dt.float32
BF16 = mybir.dt.bfloat16
BF16NP = ml_dtypes.bfloat16

# Problem constants (hardcoded per harness contract)
B, N, C, CV, H = 4, 2048, 768, 512, 8
DH, DV = C // H, CV // H          # 96, 64
NH = 4                            # heads per core
NDO = NH * DH                     # 384 projected q/k dims per core
NDV = NH * DV                     # 256 projected v dims per core
SCALE = float((256 // 8) ** (-0.5))
N_CORES = 8


def build_nc(NT: int = N):
    """Build the per-core Bass program. NT = token count (param for small sims)."""
    NJT = NT // 128               # j tiles
    NIT = NT // 128               # i tiles
    assert NT % 512 == 0
    ICH = 1024 if NT % 1024 == 0 else 512   # exp chunk width

    NCT = C // 128                # 6 c tiles
    NVT = CV // 128               # 4 cv tiles
    WALL = 2 * NCT * NDO + NVT * NDV
    nc = bacc_mod.Bacc()
    # all inputs host-packed to the exact SBUF image: [128, k*W] where
    # partition p row-interleaves rows {p, 128+p, ...} of the logical tensor
    qT = nc.declare_dram_parameter("qT", [128, NCT * NT], BF16, isOutput=False)
    kT = nc.declare_dram_parameter("kT", [128, NCT * NT], BF16, isOutput=False)
    vT = nc.declare_dram_parameter("vT", [128, NVT * NT], BF16, isOutput=False)
    mT = nc.declare_dram_parameter("mT", [128, NJT * NT], BF16, isOutput=False)
    wall = nc.declare_dram_parameter("wall", [128, WALL], BF16, isOutput=False)
    out = nc.declare_dram_parameter("out", [128, NIT * NDV], F32, isOutput=True)

    with ExitStack() as top:
        tc = top.enter_context(tile.TileContext(nc))
        persist = top.enter_context(tc.tile_pool(name="persist", bufs=1))

        # ---- masks (transposed) resident in SBUF. DMAs are issued in
        # chunks AFTER w/q/k/v on the same sync ring (FIFO = priority).
        mt_all = persist.tile([128, NJT, NT], BF16, tag="mt", name="mt_all")
        mt_tiles = [mt_all[:, jt, :] for jt in range(NJT)]

        # summ8[i] = 8 * sum_j m[i, j]; filled during head 0 via a ones-row
        # matmul over mT accumulated into ut_ps rows 96 (see below).
        summ8 = persist.tile([128, NIT], F32, tag="summ8", name="summ8")

        # ---- projections ----
        qpt = [persist.tile([DH, NT], BF16, tag=f"qpt{h}", name=f"qpt{h}") for h in range(NH)]
        kpt = [persist.tile([DH, NT], BF16, tag=f"kpt{h}", name=f"kpt{h}") for h in range(NH)]
        vp = persist.tile([128, NJT, NDV], BF16, tag="vp", name="vp")

        with ExitStack() as projctx:
            qkv_pool = projctx.enter_context(tc.tile_pool(name="qkv", bufs=1))
            w_pool = projctx.enter_context(tc.tile_pool(name="w", bufs=1))
            ppsum = projctx.enter_context(
                tc.tile_pool(name="ppsum", bufs=4, space="PSUM"))

            def load_whole(dram, n_tiles, width, tag, split=1):
                t = qkv_pool.tile([128, n_tiles, width], BF16, tag=tag, name=tag)
                w2 = n_tiles * width
                for s in range(split):
                    a, b = s * w2 // split, (s + 1) * w2 // split
                    nc.sync.dma_start(
                        out=t.rearrange("p a n -> p (a n)")[:, a:b],
                        in_=dram[:, a:b])
                return [t[:, i, :] for i in range(n_tiles)]

            w_sb = w_pool.tile([128, WALL], BF16, tag="wall", name="w_sb")
            nc.sync.dma_start(out=w_sb, in_=wall[:, :])
            wqts = [w_sb[:, i * NDO:(i + 1) * NDO] for i in range(NCT)]
            wkts = [w_sb[:, (NCT + i) * NDO:(NCT + i + 1) * NDO]
                    for i in range(NCT)]
            wv0 = 2 * NCT * NDO
            wvts = [w_sb[:, wv0 + i * NDV:wv0 + (i + 1) * NDV]
                    for i in range(NVT)]
            qts = load_whole(qT, NCT, NT, "q", split=2)
            kts = load_whole(kT, NCT, NT, "k", split=2)
            vts = load_whole(vT, NVT, NT, "v")
            for s in range(4):
                a, b = s * NJT // 4, (s + 1) * NJT // 4
                nc.sync.dma_start(
                    out=mt_all[:, a:b, :],
                    in_=mT[:, a * NT:b * NT])

            # QpT/KpT: out[dh, tok-chunk] = W_slice @ x.T
            # ci outer so 4 chunks share one stationary load per (h, ci)
            NCH = NT // 512
            for h in range(NH):
                for dst, wts, xts in ((qpt, wqts, qts), (kpt, wkts, kts)):
                    pss = [ppsum.tile([DH, 512], F32, tag="pp", name="pp")
                           for _ in range(NCH)]
                    for ci in range(NCT):
                        for ch in range(NCH):
                            nc.tensor.matmul(
                                pss[ch],
                                lhsT=wts[ci][:, h * DH:(h + 1) * DH],
                                rhs=xts[ci][:, ch * 512:(ch + 1) * 512],
                                start=(ci == 0), stop=(ci == NCT - 1),
                            )
                    for ch in range(NCH):
                        nc.vector.tensor_copy(
                            out=dst[h][:, ch * 512:(ch + 1) * 512], in_=pss[ch])

            # Vp natural: out[tok-tile, dv_all]
            for jt in range(NJT):
                ps = ppsum.tile([128, NDV], F32, tag="pv", name="pv")
                for ci in range(NVT):
                    nc.tensor.matmul(
                        ps,
                        lhsT=vts[ci][:, jt * 128:(jt + 1) * 128],
                        rhs=wvts[ci],
                        start=(ci == 0), stop=(ci == NVT - 1),
                    )
                nc.vector.tensor_copy(out=vp[:, jt, :], in_=ps)

        # ---- attention ----
        ones = persist.tile([128, 1], BF16, tag="ones", name="ones")
        nc.vector.memset(ones, 1.0)
        ident = persist.tile([128, 128], F32, tag="ident", name="ident")
        make_identity(nc, ident)
        x_sb = [persist.tile([128, NDV], F32, tag=f"x{it}", name=f"x{it}") for it in range(NIT)]

        spsum = top.enter_context(tc.tile_pool(name="spsum", bufs=2, space="PSUM"))
        utpsum = top.enter_context(tc.tile_pool(name="utpsum", bufs=2, space="PSUM"))
        streams = top.enter_context(tc.tile_pool(name="streams", bufs=3))
        utsb_pool = top.enter_context(tc.tile_pool(name="utsb", bufs=2))
        small = top.enter_context(tc.tile_pool(name="small", bufs=4))

        IH = min(1024, NT)            # i-half width
        NHF = NT // IH                # number of i-halves
        NIT_H = IH // 128             # i tiles per half
        summr = persist.tile([1, NT], F32, tag="summr", name="summr")

        def emit_summ_phase():
            # summ8 = 8 * row-sums of masks via ones-matmul over mT columns
            for half in range(NT // ICH):
                sm_ps = utpsum.tile([1, ICH], F32, tag="ut", name="sm_ps")
                for jt in range(NJT):
                    for q2 in range(ICH // 512):
                        nc.tensor.matmul(
                            sm_ps[:, q2 * 512:(q2 + 1) * 512],
                            lhsT=ones,
                            rhs=mt_tiles[jt][:, half * ICH + q2 * 512:
                                             half * ICH + (q2 + 1) * 512],
                            start=(jt == 0), stop=(jt == NJT - 1),
                            skip_group_check=True,
                        )
                nc.vector.tensor_copy(
                    out=summr[:, half * ICH:(half + 1) * ICH], in_=sm_ps)
            sumn_ps = utpsum.tile([128, NIT], F32, tag="ut", name="sumn_ps")
            for it in range(NIT):
                nc.tensor.transpose(
                    out=sumn_ps[:, it:it + 1],
                    in_=summr[:, it * 128:(it + 1) * 128],
                    identity=ident[0:1, 0:1],
                )
            nc.vector.tensor_scalar_mul(summ8, sumn_ps, float(H))

        for h in range(NH):
            for ihalf in range(NHF):
                i0 = ihalf * IH
                ut_ps = utpsum.tile([128, IH], F32, tag="ut", name="ut")
                eacc = streams.tile([128, IH], BF16, tag="esum", name="eacc", bufs=2)
                for jt in range(NJT):
                    expst = streams.tile([128, IH], BF16, tag="expst", name="expst")
                    s_ps = spsum.tile([128, IH], F32, tag="s", name="s_ps")
                    for q2 in range(IH // 512):
                        nc.tensor.matmul(
                            s_ps[:, q2 * 512:(q2 + 1) * 512],
                            lhsT=kpt[h][:, jt * 128:(jt + 1) * 128],
                            rhs=qpt[h][:, i0 + q2 * 512: i0 + (q2 + 1) * 512],
                            start=True, stop=True,
                        )
                    nc.scalar.activation(
                        out=expst, in_=s_ps,
                        func=mybir.ActivationFunctionType.Exp, scale=SCALE,
                    )
                    bsb = streams.tile([128, IH], BF16, tag="b", name="bsb")
                    nc.vector.tensor_tensor(
                        out=bsb, in0=expst, in1=mt_tiles[jt][:, i0:i0 + IH],
                        op=mybir.AluOpType.mult)
                    # running per-partition exp sum on DVE (bf16, 4x mode);
                    # the final contraction over j%128 happens once per half
                    if jt == 0:
                        nc.vector.tensor_copy(out=eacc, in_=expst)
                    else:
                        nc.vector.tensor_tensor(
                            out=eacc, in0=eacc, in1=expst,
                            op=mybir.AluOpType.add)
                    first, last = (jt == 0), (jt == NJT - 1)
                    for ic in range(IH // 512):
                        sl = slice(ic * 512, (ic + 1) * 512)
                        nc.tensor.matmul(
                            ut_ps[0:DV, sl],
                            lhsT=vp[:, jt, h * DV:(h + 1) * DV],
                            rhs=bsb[:, sl],
                            start=first, stop=last, skip_group_check=True,
                        )
                for ic in range(IH // 512):
                    sl = slice(ic * 512, (ic + 1) * 512)
                    nc.tensor.matmul(
                        ut_ps[DV:DV + 1, sl],
                        lhsT=ones,
                        rhs=eacc[:, sl],
                        start=True, stop=True, skip_group_check=True,
                    )

                if h == 0 and ihalf == 0:
                    emit_summ_phase()

                # epilogue for this i-half
                ut_sb = utsb_pool.tile([DV + 1, IH], F32, tag="utsb", name="utsb")
                nc.vector.tensor_copy(out=ut_sb, in_=ut_ps[0:DV + 1, :])
                gsz = min(4, NIT_H)
                for g in range(NIT_H // gsz):
                    un_ps = utpsum.tile([128, gsz, DV + 1], F32, tag="ut", name="un_ps")
                    its = [ihalf * NIT_H + g * gsz + t for t in range(gsz)]
                    for t in range(gsz):
                        nc.tensor.transpose(
                            out=un_ps[:, t, :],
                            in_=ut_sb[:, (g * gsz + t) * 128:(g * gsz + t + 1) * 128],
                            identity=ident[0:DV + 1, 0:DV + 1],
                        )
                    un_sb = small.tile([128, gsz, DV + 1], F32,
                                       tag="unsb", name="un_sb")
                    nc.vector.tensor_copy(out=un_sb, in_=un_ps)
                    den = small.tile([128, gsz], F32, tag="den", name="den")
                    rec = small.tile([128, gsz], F32, tag="rec", name="rec")
                    nc.vector.tensor_tensor(
                        out=den, in0=un_sb[:, :, DV],
                        in1=summ8[:, its[0]:its[0] + gsz], op=mybir.AluOpType.mult)
                    nc.vector.reciprocal(rec, den)
                    for t in range(gsz):
                        nc.vector.tensor_scalar_mul(
                            x_sb[its[t]][:, h * DV:(h + 1) * DV],
                            un_sb[:, t, 0:DV],
                            rec[:, t:t + 1],
                        )
                    if h == NH - 1:
                        for t in range(gsz):
                            it = its[t]
                            nc.sync.dma_start(
                                out=out[:, it * NDV:(it + 1) * NDV],
                                in_=x_sb[it])

    nc.finalize()
    return nc


_NC_CACHE: dict = {}


def get_nc(NT: int = N):
    if NT not in _NC_CACHE:
        _NC_CACHE[NT] = build_nc(NT)
    return _NC_CACHE[NT]


def _pack(x):
    """[k*128, W] -> [128, k*W]: partition p holds rows {p, 128+p, ...}."""
    k = x.shape[0] // 128
    return x.reshape(k, 128, -1).transpose(1, 0, 2).reshape(128, -1)


def pack_core(qb, kb, vb, mb, wq_s, wk_s, wv_s):
    """Build one core's packed bf16 input dict from raw (transposed) slices."""

    def bf(x):
        return np.ascontiguousarray(_pack(x.astype(np.float32).astype(BF16NP)))

    wall = np.concatenate(
        [_pack(wq_s.astype(np.float32).astype(BF16NP)),
         _pack(wk_s.astype(np.float32).astype(BF16NP)),
         _pack(wv_s.astype(np.float32).astype(BF16NP))], axis=1)
    return {
        "qT": bf(qb), "kT": bf(kb), "vT": bf(vb), "mT": bf(mb),
        "wall": np.ascontiguousarray(wall),
    }


def unpack_out(res, NT=N):
    """[128, NIT*NDV] -> [NT, NDV]."""
    nit = NT // 128
    return res.reshape(128, nit, NDV).transpose(1, 0, 2).reshape(NT, NDV)


def make_in_maps(q, k, v, masks, Wq, Wk, Wv):
    """Host-side shard + layout prep. Returns per-core input dicts."""
    in_maps = []
    for c in range(N_CORES):
        b, hg = c // 2, c % 2
        in_maps.append(pack_core(
            q[b].T, k[b].T, v[b].T, masks[b].T,
            Wq[hg * NDO:(hg + 1) * NDO, :].T,
            Wk[hg * NDO:(hg + 1) * NDO, :].T,
            Wv[hg * NDV:(hg + 1) * NDV, :].T,
        ))
    return in_maps


def _reset_device():
    import ctypes
    try:
        lib = ctypes.CDLL("/opt/axon/libaxon_pjrt.so")
        lib.axon_reset.restype = ctypes.c_int64
        lib.axon_reset()
    except Exception:
        pass


def kernel(q, k, v, masks, Wq, Wk, Wv, **_unused):
    from concourse.bass_utils import run_bass_kernel_spmd

    q, k, v, masks = (np.asarray(x) for x in (q, k, v, masks))
    Wq, Wk, Wv = (np.asarray(x) for x in (Wq, Wk, Wv))

    nc = get_nc(N)
    in_maps = make_in_maps(q, k, v, masks, Wq, Wk, Wv)
    try:
        res = run_bass_kernel_spmd(
            nc, in_maps, core_ids=list(range(N_CORES))).results
    except Exception:
        # wedged accelerator (e.g. NRT_EXEC_UNIT_UNRECOVERABLE) — reset + retry
        _reset_device()
        res = run_bass_kernel_spmd(
            nc, in_maps, core_ids=list(range(N_CORES))).results

    full = np.empty((B, N, CV), np.float32)
    for c in range(N_CORES):
        b, hg = c // 2, c % 2
        full[b][:, hg * NDV:(hg + 1) * NDV] = unpack_out(res[c]["out"])
    return full

